# revision 4
# baseline (speedup 1.0000x reference)
"""CrossAttention TRN2 kernel: B=8 data-parallel over 8 NeuronCores.

Per core (one batch element):
  q = LN(xq @ Wq + bq) * gq + betaq          [2048, 512]
  k = LN(xkv @ Wk + bk) * gk + betak         [2048, 512]
  v = LN(xkv @ Wv + bv) * gv + betav         [2048, 512]
  S = k @ q.T                                 [2048, 2048]
  P = softmax(S, axis=-1)
  out = P @ v + xq                            [2048, 512]

All matmuls in fp16 (1 cyc/row on PE, enough mantissa for the near-one-hot
softmax), accumulation + softmax/LN statistics in fp32.
"""

import numpy as np

import concourse.bass as bass
import concourse.tile as tile
from concourse import mybir, masks
from concourse.bass_utils import run_bass_kernel_spmd
from concourse.vector_clock import ScopedClock
import bass_rust

F32 = mybir.dt.float32
F16 = mybir.dt.float16
AF = mybir.ActivationFunctionType
ALU = mybir.AluOpType

L = 2048
DQ = 512
DKV = 768
HID = 512
NB = L // 128          # 16 l-blocks
NJQ = DQ // 128        # 4 contraction chunks for Wq
NJK = DKV // 128       # 6 for Wk/Wv
NCORES = 8
EPS = 1e-5


def _patched_drain_and_barrier(self, tick_clock, wait_clock):
    # walrus in this container rejects any instruction carrying >1 sem wait
    # ("Too many sync wait commands"); split the exit-drain waits across
    # single-wait NoOps on the sync engine (strict FIFO => same semantics).
    nop1 = self.nc.sync.nop(nofuse=True, hint="pre_drain_w0")
    wait_clock.add_sem_waits(nop1.ins, ScopedClock({None: tick_clock.global_clock}))
    waits = list(nop1.ins.sync_info.on_wait)
    if len(waits) > 1:
        nop1.ins.sync_info = bass_rust.SyncInfo(on_wait=waits[:1], on_update=[])
        for w in waits[1:]:
            nop = self.nc.sync.nop(nofuse=True, hint="pre_drain_w")
            nop.ins.sync_info = bass_rust.SyncInfo(on_wait=[w], on_update=[])
    self.nc.sync.drain()
    self.nc.all_engine_barrier()
    assert self.sems is not None
    popped = self.nc._tile_sem_poison_stack.pop()
    assert popped is self._sem_poison
    self.nc.clear_and_free_semaphores(list(self.sems.allocated().values()))
    self.nc.all_engine_barrier()


tile.TileContext._drain_and_barrier = _patched_drain_and_barrier


def _split_multiwait_instructions(nc):
    """walrus here rejects >1 sem wait per instruction; hoist extras onto
    fresh single-wait NoOps inserted just before, same engine (strict FIFO
    per engine => identical semantics)."""
    n = 0
    for f in nc.m.functions:
        for bb in f.blocks:
            insts = list(bb.instructions)
            new = []
            changed = False
            for ins in insts:
                si = ins.sync_info
                waits = list(si.on_wait) if si is not None else []
                if len(waits) > 1:
                    changed = True
                    for w in waits[:-1]:
                        n += 1
                        nop = mybir.InstNoOp(name=f"mwsplit_{n}")
                        nop.engine = ins.engine
                        nop.sync_info = bass_rust.SyncInfo(on_wait=[w], on_update=[])
                        new.append(nop)
                    ins.sync_info = bass_rust.SyncInfo(
                        on_wait=[waits[-1]], on_update=list(si.on_update))
                new.append(ins)
            if changed:
                bb.instructions = new
    return n


def _build():
    nc = bass.Bass("TRN2", target_bir_lowering=False, debug=False,
                   num_devices=NCORES)

    xq_d = nc.dram_tensor("xq", [L, DQ], F32, kind="ExternalInput").ap()
    xkv_d = nc.dram_tensor("xkv", [L, DKV], F32, kind="ExternalInput").ap()
    wq_d = nc.dram_tensor("Wq", [DQ, HID], F32, kind="ExternalInput").ap()
    wk_d = nc.dram_tensor("Wk", [DKV, HID], F32, kind="ExternalInput").ap()
    wv_d = nc.dram_tensor("Wv", [DKV, DQ], F32, kind="ExternalInput").ap()
    vecs = {}
    for name in ["bq", "gq", "betaq", "bk", "gk", "betak", "bv", "gv", "betav"]:
        vecs[name] = nc.dram_tensor(name, [1, HID], F32, kind="ExternalInput").ap()
    out_d = nc.dram_tensor("out", [L, DQ], F32, kind="ExternalOutput").ap()

    with tile.TileContext(nc) as tc:
        _emit(tc, nc, xq_d, xkv_d, wq_d, wk_d, wv_d, vecs, out_d)
    _split_multiwait_instructions(nc)
    return nc


def _emit(tc, nc, xq_d, xkv_d, wq_d, wk_d, wv_d, vecs, out_d):
    import contextlib
    ctx = contextlib.ExitStack()
    with ctx:
        # ---- persistent tensors (bufs=1 pools) ----
        persist = ctx.enter_context(tc.tile_pool(name="persist", bufs=1))
        xqT = persist.tile([128, NJQ, L], F16, tag="xqT")      # 2 MB
        xkvT = persist.tile([128, NJK, L], F16, tag="xkvT")    # 3 MB
        wq_s = persist.tile([128, NJQ, HID], F16, tag="wq")
        wk_s = persist.tile([128, NJK, HID], F16, tag="wk")
        wv_s = persist.tile([128, NJK, DQ], F16, tag="wv")
        qT = persist.tile([128, NJQ, L], F16, tag="qT")        # 2 MB
        kT = persist.tile([128, NJQ, L], F16, tag="kT")        # 2 MB
        v_s = persist.tile([128, NB, DQ], F16, tag="v")        # 2 MB
        ident = persist.tile([128, 128], F16, tag="ident")
        ones_col = persist.tile([1, 128], F16, tag="ones")
        repl = {n: persist.tile([128, HID], F16, tag=f"repl_{n}", name=f"repl_{n}")
                for n in ["gq", "betaq", "gk", "betak", "gv", "betav"]}
        brow = {n: persist.tile([1, HID], F16, tag=f"brow_{n}", name=f"brow_{n}")
                for n in ["bq", "bk", "bv"]}

        masks.make_identity(nc, ident[:])
        nc.gpsimd.memset(ones_col[:], 1.0)

        # ---- small staging pools ----
        vrow_pool = ctx.enter_context(tc.tile_pool(name="vrow", bufs=2))
        setup_ctx = contextlib.ExitStack()
        psum_b = setup_ctx.enter_context(tc.tile_pool(name="psum_b", bufs=2, space="PSUM"))

        # bias rows: load f32, cast to f16 rows
        for n in ["bq", "bk", "bv"]:
            st = vrow_pool.tile([1, HID], F32, tag="vstage")
            nc.sync.dma_start(st[:], vecs[n][:])
            nc.vector.tensor_copy(brow[n][:], st[:])
        # g/beta: load, cast, broadcast to 128 partitions via K=1 matmul
        for n in ["gq", "betaq", "gk", "betak", "gv", "betav"]:
            st = vrow_pool.tile([1, HID], F32, tag="vstage")
            nc.sync.dma_start(st[:], vecs[n][:])
            row16 = vrow_pool.tile([1, HID], F16, tag="vrow16")
            nc.vector.tensor_copy(row16[:], st[:])
            pb = psum_b.tile([128, HID], F32)
            nc.tensor.matmul(pb[:], ones_col[:], row16[:], start=True, stop=True)
            nc.scalar.activation(repl[n][:], pb[:], AF.Copy)

        # ---- weights: load + cast ----
        wstage = ctx.enter_context(tc.tile_pool(name="wstage", bufs=3))
        for (wd, ws, nj) in [(wq_d, wq_s, NJQ), (wk_d, wk_s, NJK), (wv_d, wv_s, NJK)]:
            for j in range(nj):
                st = wstage.tile([128, HID], F32, tag="wstage")
                nc.sync.dma_start(st[:], wd[128 * j:128 * (j + 1), :])
                nc.vector.tensor_copy(ws[:, j, :], st[:])

        # ---- X load / cast / transpose ----
        xstage = ctx.enter_context(tc.tile_pool(name="xstage", bufs=3))
        psum_t = setup_ctx.enter_context(tc.tile_pool(name="psum_t", bufs=2, space="PSUM"))
        for (xd, xT, nj, width) in [(xq_d, xqT, NJQ, DQ), (xkv_d, xkvT, NJK, DKV)]:
            for i in range(NB):
                st = xstage.tile([128, width], F32, tag="xstage")
                nc.sync.dma_start(st[:], xd[128 * i:128 * (i + 1), :])
                st16 = xstage.tile([128, width], F16, tag="xstage16")
                nc.vector.tensor_copy(st16[:], st[:])
                pt = psum_t.tile([128, nj * 128], F16, tag="ptrans")
                for j in range(nj):
                    nc.tensor.transpose(pt[:, 128 * j:128 * (j + 1)],
                                        st16[:, 128 * j:128 * (j + 1)], ident[:])
                dst = xT[:, :, 128 * i:128 * (i + 1)]  # [128, nj, 128]
                src = pt[:].rearrange("p (a b) -> p a b", a=nj)
                nc.scalar.activation(dst, src, AF.Copy)

        setup_ctx.close()

        # ---- projections + LN per l-block ----
        proj_ctx = contextlib.ExitStack()
        psum_h = proj_ctx.enter_context(tc.tile_pool(name="psum_h", bufs=4, space="PSUM"))
        psum_qk = proj_ctx.enter_context(tc.tile_pool(name="psum_qk", bufs=2, space="PSUM"))
        hstage = ctx.enter_context(tc.tile_pool(name="hstage", bufs=3))
        stat_pool = ctx.enter_context(tc.tile_pool(name="stats", bufs=4))

        for i in range(NB):
            li = slice(128 * i, 128 * (i + 1))
            ph = {}
            for (nm, xT, nj, ws, bn) in [("q", xqT, NJQ, wq_s, "bq"),
                                         ("k", xkvT, NJK, wk_s, "bk"),
                                         ("v", xkvT, NJK, wv_s, "bv")]:
                p = psum_h.tile([128, HID], F32, tag="ph")
                for j in range(nj):
                    nc.tensor.matmul(p[:], xT[:, j, li], ws[:, j, :],
                                     start=(j == 0), stop=False)
                nc.tensor.matmul(p[:], ones_col[:], brow[bn][:],
                                 start=False, stop=True)
                ph[nm] = p

            # stats: cols 0..2 sums, 3..5 sumsq (q,k,v)
            st = stat_pool.tile([128, 16], F32, tag="st")
            junk = hstage.tile([128, HID], F16, tag="junk")
            for idx, nm in enumerate(["q", "k", "v"]):
                nc.vector.tensor_reduce(st[:, idx:idx + 1], ph[nm][:],
                                        mybir.AxisListType.X, ALU.add)
                nc.scalar.activation(junk[:], ph[nm][:], AF.Square,
                                     accum_out=st[:, 3 + idx:4 + idx])
            mu = st[:, 6:9]
            ms = st[:, 9:12]
            var = st[:, 12:15]
            nc.vector.tensor_scalar_mul(mu, st[:, 0:3], 1.0 / HID)
            nc.vector.tensor_scalar_mul(ms, st[:, 3:6], 1.0 / HID)
            # var = ms - mu*mu  (one op: (mu * -mu) + ms)  -- need mu*mu first
            nc.vector.tensor_tensor(var, mu, mu, ALU.mult)
            nc.vector.tensor_tensor(var, ms, var, ALU.subtract)
            nc.vector.tensor_scalar_add(var, var, EPS)
            lnv = stat_pool.tile([128, 3], F32, tag="lnv")
            nc.scalar.activation(lnv[:], var, AF.Ln)
            rs = stat_pool.tile([128, 3], F32, tag="rs")
            nc.scalar.activation(rs[:], lnv[:], AF.Exp, scale=-0.5)
            nmrs = stat_pool.tile([128, 3], F32, tag="nmrs")
            nc.vector.scalar_tensor_tensor(nmrs[:], mu, -1.0, rs[:],
                                           ALU.mult, ALU.mult)

            # apply LN -> f16, then g/beta
            hq = hstage.tile([128, HID], F16, tag="hq")
            hk = hstage.tile([128, HID], F16, tag="hk")
            for nm, dst, gr, br_ in [("q", hq[:], repl["gq"], repl["betaq"]),
                                     ("k", hk[:], repl["gk"], repl["betak"]),
                                     ("v", v_s[:, i, :], repl["gv"], repl["betav"])]:
                idx = {"q": 0, "k": 1, "v": 2}[nm]
                nc.scalar.activation(dst, ph[nm][:], AF.Identity,
                                     bias=nmrs[:, idx:idx + 1],
                                     scale=rs[:, idx:idx + 1])
                nc.vector.tensor_tensor(dst, dst, gr[:], ALU.mult)
                nc.vector.tensor_tensor(dst, dst, br_[:], ALU.add)

            # transpose q,k tiles into one fp16 psum bank
            pqk = psum_qk.tile([128, 1024], F16, tag="pqk")
            for j in range(NJQ):
                nc.tensor.transpose(pqk[:, 128 * j:128 * (j + 1)],
                                    hq[:, 128 * j:128 * (j + 1)], ident[:])
            for j in range(NJQ):
                nc.tensor.transpose(pqk[:, 512 + 128 * j:512 + 128 * (j + 1)],
                                    hk[:, 128 * j:128 * (j + 1)], ident[:])
            nc.vector.tensor_copy(qT[:, :, li],
                                  pqk[:, 0:512].rearrange("p (a b) -> p a b", a=NJQ))
            nc.vector.tensor_copy(kT[:, :, li],
                                  pqk[:, 512:1024].rearrange("p (a b) -> p a b", a=NJQ))

        proj_ctx.close()

        # ---- attention per l-block ----
        psum_s = ctx.enter_context(tc.tile_pool(name="psum_s", bufs=1, space="PSUM"))
        psum_pt = ctx.enter_context(tc.tile_pool(name="psum_pt", bufs=2, space="PSUM"))
        psum_o = ctx.enter_context(tc.tile_pool(name="psum_o", bufs=2, space="PSUM"))
        p_pool = ctx.enter_context(tc.tile_pool(name="p_sb", bufs=2))
        pt_pool = ctx.enter_context(tc.tile_pool(name="pt_sb", bufs=2))
        att_small = ctx.enter_context(tc.tile_pool(name="att_small", bufs=3))
        out_pool = ctx.enter_context(tc.tile_pool(name="out_sb", bufs=3))
        xq_res_pool = ctx.enter_context(tc.tile_pool(name="xq_res", bufs=3))

        for i in range(NB):
            li = slice(128 * i, 128 * (i + 1))
            ps = psum_s.tile([128, 4, 512], F32, tag="ps")
            for s in range(4):
                for j in range(NJQ):
                    nc.tensor.matmul(ps[:, s, :], kT[:, j, li],
                                     qT[:, j, 512 * s:512 * (s + 1)],
                                     start=(j == 0), stop=(j == NJQ - 1))
            negmax = att_small.tile([128, 1], F32, tag="negmax")
            nc.vector.tensor_reduce(negmax[:], ps[:], mybir.AxisListType.XY,
                                    ALU.max, negate=True)
            rowsum = att_small.tile([128, 1], F32, tag="rowsum")
            p_sb = p_pool.tile([128, L], F16, tag="p")
            nc.scalar.activation(p_sb[:].rearrange("p (a b) -> p a b", a=4),
                                 ps[:], AF.Exp, bias=negmax[:],
                                 accum_out=rowsum[:])
            inv = att_small.tile([128, 1], F32, tag="inv")
            nc.vector.reciprocal(inv[:], rowsum[:])

            pt_sb = pt_pool.tile([128, NB, 128], F16, tag="pt")
            for half in range(2):
                pp = psum_pt.tile([128, 1024], F16, tag="ppt")
                for t in range(8):
                    tt = 8 * half + t
                    nc.tensor.transpose(pp[:, 128 * t:128 * (t + 1)],
                                        p_sb[:, 128 * tt:128 * (tt + 1)], ident[:])
                nc.vector.tensor_copy(
                    pt_sb[:, 8 * half:8 * (half + 1), :],
                    pp[:].rearrange("p (a b) -> p a b", a=8))

            po = psum_o.tile([128, DQ], F32, tag="po")
            for t in range(NB):
                nc.tensor.matmul(po[:], pt_sb[:, t, :], v_s[:, t, :],
                                 start=(t == 0), stop=(t == NB - 1))

            xq_res = xq_res_pool.tile([128, DQ], F32, tag="xq_res")
            nc.sync.dma_start(xq_res[:], xq_d[li, :])
            out_t = out_pool.tile([128, DQ], F32, tag="out_t")
            nc.vector.scalar_tensor_tensor(out_t[:], po[:], inv[:], xq_res[:],
                                           ALU.mult, ALU.add)
            nc.sync.dma_start(out_d[li, :], out_t[:])


_NC_CACHE = {}


def _get_nc():
    if "nc" not in _NC_CACHE:
        _NC_CACHE["nc"] = _build()
    return _NC_CACHE["nc"]


def _in_maps(inputs):
    maps = []
    for b in range(NCORES):
        m = {
            "xq": np.ascontiguousarray(inputs["query_sequence"][b], dtype=np.float32),
            "xkv": np.ascontiguousarray(inputs["key_value_sequence"][b], dtype=np.float32),
            "Wq": np.asarray(inputs["Wq"], dtype=np.float32),
            "Wk": np.asarray(inputs["Wk"], dtype=np.float32),
            "Wv": np.asarray(inputs["Wv"], dtype=np.float32),
        }
        for n in ["bq", "gq", "betaq", "bk", "gk", "betak", "bv", "gv", "betav"]:
            m[n] = np.asarray(inputs[n], dtype=np.float32).reshape(1, HID)
        maps.append(m)
    return maps


def run(trace=False, **inputs):
    nc = _get_nc()
    res = run_bass_kernel_spmd(nc, _in_maps(inputs), list(range(NCORES)),
                               trace=trace)
    out = np.stack([res.results[b]["out"] for b in range(NCORES)], axis=0)
    return out.astype(np.float32), res


def kernel(**inputs) -> np.ndarray:
    out, _ = run(trace=False, **inputs)
    return out
